# revision 34
# baseline (speedup 1.0000x reference)
"""Trainium2 Bass kernel for nn_CBAMSpaceMask (CBAM spatial mask over T timestep blocks).

Math per timestep block t (3 channels):
  mx_c = maxpool3x3(x_c)          (stride 1, -inf pad == replicate pad)
  av_c = avgpool3x3(x_c)/9        (zero pad, count_include_pad)
  y_t  = sum_c wM_c * mx_c + wA_c * av_c + b   (3x3 conv, zero pad)
  out[3t+c] = sigmoid(leakyrelu(y_t))          (broadcast over c)

v2 design (per core = 1 batch element, pure data parallel over batch):
  - input loaded from HBM exactly once (f32->bf16 cast DMA, gpsimd SWDGE);
    row-shifted U/D tiles for the vertical max built by SBUF->SBUF
    partition-shifted DMA copies + clamp edges (edge pool rows have zero
    conv coefficients; clamps only keep values finite)
  - groups of 12 planes (= 4 timesteps); tiles [128, 12, 256] bf16
  - pools: vertical max (2 DVE ops), horizontal 3-tap max (2 DVE ops),
    horizontal box sum for the avg path (2 gpsimd ops); vertical box sum
    folded into the conv operator (op @ Bv)
  - conv: banded-Toeplitz matmuls on PE, timestep-PAIRED rhs (planes
    {c, c+3} via stride-3 free slice) -> N=2x254, psum [124, 2, 256]
  - last-8-rows chunk: rows 246..255 packed per-group at partition bases
    0/32/64/96; kw taps folded into K=30 stacked matmuls using L/R
    column-shifted copies of the pooled tiles
  - epilogue: ACT Identity(+bias) -> DVE leaky=max(v,.01v) -> ACT Sigmoid
    -> broadcast (0-stride) DMA writes each mask to its 3 channel planes
"""
import sys

sys.path.insert(0, "/opt/trn_rl_repo")

import numpy as np
import ml_dtypes
from contextlib import ExitStack

import concourse.bass as bass
import concourse.tile as tile
from concourse import bacc, mybir
from concourse.bass_utils import run_bass_kernel_spmd

F32 = mybir.dt.float32
BF16 = mybir.dt.bfloat16

B, CTOT, H, W = 8, 48, 256, 256
T = 16
N_CORES = 8
NGRP = 4            # groups of 12 planes = 4 timesteps
GP = 12             # planes per group
# main chunk geometry: y rows [m0,m1) from x rows [r0,r1)
SUBS = [(0, 124, 0, 128), (124, 248, 122, 250)]
C2 = (248, 256, 246, 256)   # last-8-rows chunk
NMAIN = 2 * 3 * 3 * 2       # path, c, kw, sub
NC2 = 2 * 3                 # path, c (kw folded into K=30)
NMAT = NMAIN + NC2

_cache = {}


def _build_stack(conv_w):
    """lhsT stack [128, NMAT, 128] bf16.

    mats 0..35: main-sub ops, idx = ((path*3 + c)*3 + kw)*2 + sub,
      lhsT = op[m0:m1, r0:r1].T  ([K=128, M=124])
    mats 36..41: chunk-2 stacked ops, idx = 36 + path*3 + c,
      [K=30, M=8]: K blocks of 10 rows for kw = 1 (center), 0, 2,
      replicated at partition bases 0/32/64/96.
    """
    w = conv_w[0].astype(np.float64)  # [6, 3, 3]
    Bv = np.zeros((H, H))
    for i in (-1, 0, 1):
        Bv += np.eye(H, k=i)
    stack = np.zeros((128, NMAT, 128), dtype=np.float64)

    def band_op(path, c, kw):
        op = np.zeros((H, H))
        k2d = w[2 * c] if path == 0 else w[2 * c + 1]
        for kh in range(3):
            op += k2d[kh, kw] * np.eye(H, k=kh - 1)
        if path == 1:
            op = (op @ Bv) / 9.0
        return op

    for path in range(2):
        for c in range(3):
            for kw in range(3):
                op = band_op(path, c, kw)
                for sub, (m0, m1, r0, r1) in enumerate(SUBS):
                    mat = ((path * 3 + c) * 3 + kw) * 2 + sub
                    lhsT = op[m0:m1, r0:r1].T  # [K, M]
                    K, M = lhsT.shape
                    stack[:K, mat, :M] = lhsT
            # chunk 2: kw-stacked [30, 8]
            mat = NMAIN + path * 3 + c
            m0, m1, r0, r1 = C2
            for kwi, kw in enumerate((1, 0, 2)):
                lhsT = band_op(path, c, kw)[m0:m1, r0:r1].T  # [10, 8]
                for base in (0, 32, 64, 96):
                    stack[base + 10 * kwi:base + 10 * kwi + 10, mat, :8] = lhsT
    return stack.astype(ml_dtypes.bfloat16)


def _mat_main(path, c, kw, sub):
    return ((path * 3 + c) * 3 + kw) * 2 + sub


def _mat_c2(path, c):
    return NMAIN + path * 3 + c


def _build_program():
    nc = bacc.Bacc("TRN2", target_bir_lowering=False, debug=False, enable_asserts=False)
    x_ap = nc.dram_tensor("x", [CTOT, H, W], F32, kind="ExternalInput").ap()
    cst_ap = nc.dram_tensor("cst", [128, NMAT, 128], BF16, kind="ExternalInput").ap()
    bias_ap = nc.dram_tensor("bias", [128, 1], F32, kind="ExternalInput").ap()
    # bf16 output: halves the output DMA volume through the software queue;
    # sigmoid outputs lie in (0,1) so bf16 quantization error (~0.4% rel) is
    # far inside the accuracy budget. Host upcasts to f32.
    out_ap = nc.dram_tensor("out", [CTOT, H, W], BF16, kind="ExternalOutput").ap()

    MAXOP = mybir.AluOpType.max
    ADDOP = mybir.AluOpType.add

    with tile.TileContext(nc) as tc, ExitStack() as ctx:
        const_pool = ctx.enter_context(tc.tile_pool(name="const", bufs=1))
        psum_pool = ctx.enter_context(tc.tile_pool(name="psum", bufs=6, space="PSUM"))
        epi_pool = ctx.enter_context(tc.tile_pool(name="epi", bufs=2))
        sg_pool = ctx.enter_context(tc.tile_pool(name="sg", bufs=10))
        t2_pool = ctx.enter_context(tc.tile_pool(name="t2", bufs=1))
        x_pool = ctx.enter_context(tc.tile_pool(name="xload", bufs=3))
        ud_pool = ctx.enter_context(tc.tile_pool(name="ud", bufs=2))
        mxbh_pool = ctx.enter_context(tc.tile_pool(name="mxbh", bufs=2))

        cst = const_pool.tile([128, NMAT, 128], BF16, tag="cst")
        nc.sync.dma_start(out=cst[:], in_=cst_ap)
        bias = const_pool.tile([128, 1], F32, tag="bias")
        nc.sync.dma_start(out=bias[:], in_=bias_ap)

        # ---- t2 tiles: rows 246..255 of group g at partitions 32g..32g+9,
        # planes 12g..12g+11 in the free dim. MX/BH are padded to 258 cols
        # (data at cols 1..256, zero pads) and also hold R/L column-shifted
        # copies at partition offsets +10 / +20 (kw-folded K=30).
        WP = W + 2
        T2X = t2_pool.tile([128, GP, W], BF16, tag="t2x")
        T2U = t2_pool.tile([128, GP, W], BF16, tag="t2u")
        T2D = t2_pool.tile([128, GP, W], BF16, tag="t2d")
        T2MX = t2_pool.tile([128, GP, WP], BF16, tag="t2mx")
        T2BH = t2_pool.tile([128, GP, WP], BF16, tag="t2bh")
        # zero-fill so gap partitions / pad columns stay finite zeros
        nc.vector.memzero(T2X[:])
        nc.vector.memzero(T2U[:])
        nc.vector.memzero(T2D[:])
        nc.vector.memzero(T2MX[:])
        nc.vector.memzero(T2BH[:])

        # Both row-subs share one tile: X[:, s] holds x rows r0s..r0s+127 of
        # the group's 12 planes (sub 0: rows 0..127, sub 1: rows 122..249).
        x_tiles = {}

        def load_x(g):
            """HBM->SBUF cast loads for group g (issue 2 groups ahead)."""
            src = x_ap[GP * g:GP * g + GP]
            X = x_pool.tile([128, 2, GP, W], BF16, tag="x")
            for sub, (m0, m1, r0, r1) in enumerate(SUBS):
                nc.gpsimd.dma_start(out=X[:, sub],
                                    in_=src[:, r0:r1, :].transpose([1, 0, 2]))
            x_tiles[g] = X

        def pools(g):
            """U/D partition-shifted copies + pools for both subs of g."""
            X = x_tiles.pop(g)
            U = ud_pool.tile([128, 2, GP, W], BF16, tag="u")
            D = ud_pool.tile([128, 2, GP, W], BF16, tag="d")
            # U[p] = X[p+1], D[p] = X[p-1]; clamps keep edge rows finite
            # (their conv coefficients are zero). gpsimd = software DGE:
            # packets spread across all 16 DMA engines (hardware DGE queues
            # pin SBUF->SBUF traffic to a single engine).
            nc.gpsimd.dma_start(out=U[0:127], in_=X[1:128])
            nc.gpsimd.dma_start(out=D[1:128], in_=X[0:127])
            nc.sync.dma_start(out=U[127:128], in_=X[127:128])
            nc.sync.dma_start(out=D[0:1], in_=X[0:1])
            # mx/bh padded: data at cols 1..256, cols 0/257 stay zero
            mx = mxbh_pool.tile([128, 2, GP, WP], BF16, tag="mx")
            bh = mxbh_pool.tile([128, 2, GP, WP], BF16, tag="bh")
            nc.vector.memset(mx[:, :, :, 0:1], 0)
            nc.vector.memset(mx[:, :, :, 257:258], 0)
            nc.vector.memset(bh[:, :, :, 0:1], 0)
            nc.vector.memset(bh[:, :, :, 257:258], 0)
            # bh first: it only needs X, so DVE proceeds while the U/D
            # shift DMAs are still in flight
            nc.vector.tensor_tensor(out=bh[:, :, :, 1:256], in0=X[:, :, :, 0:255],
                                    in1=X[:, :, :, 1:256], op=ADDOP)
            nc.vector.tensor_copy(bh[:, :, :, 256:257], X[:, :, :, 255:256])
            nc.vector.tensor_tensor(out=bh[:, :, :, 2:257], in0=bh[:, :, :, 2:257],
                                    in1=X[:, :, :, 0:255], op=ADDOP)
            # vertical 3-row max (DVE), in place into U
            vx = U
            nc.vector.tensor_tensor(out=vx[:], in0=U[:], in1=D[:], op=MAXOP)
            nc.vector.tensor_tensor(out=vx[:], in0=vx[:], in1=X[:], op=MAXOP)
            # horizontal 3-tap max (DVE) into padded mx
            nc.vector.tensor_tensor(out=mx[:, :, :, 1:256], in0=vx[:, :, :, 0:255],
                                    in1=vx[:, :, :, 1:256], op=MAXOP)
            nc.vector.tensor_copy(mx[:, :, :, 256:257], vx[:, :, :, 255:256])
            nc.vector.tensor_tensor(out=mx[:, :, :, 2:257], in0=mx[:, :, :, 2:257],
                                    in1=vx[:, :, :, 0:255], op=MAXOP)
            return mx, bh

        def load_t2(g):
            b = 32 * g
            src = x_ap[GP * g:GP * g + GP]
            m0, m1, r0, r1 = C2
            nc.gpsimd.dma_start(out=T2X[b:b + 10],
                                in_=src[:, r0:r1, :].transpose([1, 0, 2]))
            nc.sync.dma_start(out=T2U[b:b + 9], in_=T2X[b + 1:b + 10])
            nc.sync.dma_start(out=T2U[b + 9:b + 10], in_=T2X[b + 9:b + 10])
            nc.sync.dma_start(out=T2D[b + 1:b + 10], in_=T2X[b:b + 9])
            nc.sync.dma_start(out=T2D[b:b + 1], in_=T2X[b:b + 1])

        def t2_pools():
            """Pools over the whole packed tile (all 4 groups at once)."""
            vx = T2U
            nc.vector.tensor_tensor(out=vx[:], in0=T2U[:], in1=T2D[:], op=MAXOP)
            nc.vector.tensor_tensor(out=vx[:], in0=vx[:], in1=T2X[:], op=MAXOP)
            nc.vector.tensor_tensor(out=T2MX[:, :, 1:256], in0=vx[:, :, 0:255],
                                    in1=vx[:, :, 1:256], op=MAXOP)
            nc.vector.tensor_copy(T2MX[:, :, 256:257], vx[:, :, 255:256])
            nc.vector.tensor_tensor(out=T2MX[:, :, 2:257], in0=T2MX[:, :, 2:257],
                                    in1=vx[:, :, 0:255], op=MAXOP)
            nc.vector.tensor_tensor(out=T2BH[:, :, 1:256], in0=T2X[:, :, 0:255],
                                    in1=T2X[:, :, 1:256], op=ADDOP)
            nc.vector.tensor_copy(T2BH[:, :, 256:257], T2X[:, :, 255:256])
            nc.vector.tensor_tensor(out=T2BH[:, :, 2:257], in0=T2BH[:, :, 2:257],
                                    in1=T2X[:, :, 0:255], op=ADDOP)
            # R/L column-shifted copies into partition blocks +10 / +20.
            # All matmul rhs windows read cols 1..256 of their block:
            #   block +10 pairs kw=0 (needs P[w-1]): dest col j <- data col j-1
            #   block +20 pairs kw=2 (needs P[w+1]): dest col j <- data col j+1
            # Pad columns supply the conv zero padding.
            for g in range(NGRP):
                b = 32 * g
                for tl in (T2MX, T2BH):
                    nc.sync.dma_start(out=tl[b + 10:b + 20, :, 2:257],
                                      in_=tl[b:b + 10, :, 1:256])
                    nc.sync.dma_start(out=tl[b + 20:b + 30, :, 1:256],
                                      in_=tl[b:b + 10, :, 2:257])

        KW_ORDER = [(path, c, kw)
                    for c in range(3) for path in range(2) for kw in (1, 0, 2)]
        KW_ORDER.sort(key=lambda pck: 0 if pck[2] == 1 else 1)

        out_ready = []   # sigmoids surely complete: safe to issue triggers
        out_recent = []  # freshly issued sigmoids: age one checkpoint first

        def epilogue(ps, M, p0, m0, m1):
            """psum [M, 2, W] planar -> sigmoid(leaky(v + bias)); output DMA
            issue is DEFERRED (gpsimd queue is in-order: an output trigger
            waiting on its sigmoid would stall later load triggers)."""
            v = epi_pool.tile([128, 2, W], F32, tag="epiv")
            nc.scalar.activation(v[0:M], ps[0:M],
                                 mybir.ActivationFunctionType.Identity,
                                 bias=bias[0:M], scale=1.0)
            lk = epi_pool.tile([128, 2, W], F32, tag="epil")
            nc.vector.scalar_tensor_tensor(out=lk[0:M], in0=v[0:M], scalar=0.01,
                                           in1=v[0:M], op0=mybir.AluOpType.mult,
                                           op1=MAXOP)
            sg = sg_pool.tile([128, 2, W], BF16, tag="epis")
            nc.scalar.activation(sg[0:M], lk[0:M],
                                 mybir.ActivationFunctionType.Sigmoid)
            out_recent.append((sg, M, p0, m0, m1))

        def flush_outputs(final=False):
            for sg, M, p0, m0, m1 in out_ready:
                for j in range(2):
                    dst = out_ap[p0 + 3 * j:p0 + 3 * j + 3, m0:m1, :] \
                        .transpose([1, 0, 2])
                    src = sg[0:M, j:j + 1, :].to_broadcast([M, 3, W])
                    nc.gpsimd.dma_start(out=dst, in_=src)
            out_ready.clear()
            out_ready.extend(out_recent)
            out_recent.clear()
            if final and out_ready:
                flush_outputs()

        def conv_sub(g, sub, pair, mx, bh):
            """Main-chunk accumulation for timestep pair `pair` of group g.

            All 18 matmuls are full-width N=512: the kw tap s reads the
            padded rhs window cols (1+s)..(256+s), whose zero pads supply
            the conv zero padding.
            """
            m0, m1, r0, r1 = SUBS[sub]
            M, K = m1 - m0, r1 - r0
            ps = psum_pool.tile([128, 2, W], F32, tag="ps")
            n = len(KW_ORDER)
            for i, (path, c, kw) in enumerate(KW_ORDER):
                s = kw - 1
                mat = _mat_main(path, c, kw, sub)
                srcs = mx if path == 0 else bh
                pl = 6 * pair + c
                rhs = srcs[0:K, sub, pl:pl + 4:3, 1 + s:257 + s]
                nc.tensor.matmul(ps[0:M], cst[0:K, mat, 0:M], rhs,
                                 start=(i == 0), stop=(i == n - 1))
            p0 = GP * g + 6 * pair
            epilogue(ps, M, p0, m0, m1)

        def conv_c2(g, pair):
            """Last-8-rows accumulation (kw-folded, K=30) for a pair."""
            m0, m1, r0, r1 = C2
            M = m1 - m0
            b = 32 * g
            ps = psum_pool.tile([128, 2, W], F32, tag="ps")
            idx = 0
            for path in range(2):
                for c in range(3):
                    mat = _mat_c2(path, c)
                    src = T2MX if path == 0 else T2BH
                    pl = 6 * pair + c
                    rhs = src[b:b + 30, pl:pl + 4:3, 1:257]
                    nc.tensor.matmul(ps[0:M], cst[b:b + 30, mat, 0:M], rhs,
                                     start=(idx == 0), stop=(idx == NC2 - 1),
                                     tile_position=(b, 0))
                    idx += 1
            p0 = GP * g + 6 * pair
            epilogue(ps, M, p0, m0, m1)

        # ---- schedule: X loads run 2 groups ahead; pools for g+1 are issued
        # at the TOP of iteration g, ahead of the epilogue STTs in the DVE
        # queue, so they execute while PE runs group g's convs. Output
        # triggers age through two checkpoints before issue so no gpsimd
        # trigger ever waits on an incomplete producer.
        # X(0) then immediately pools(0): the U/D shift packets enqueue ahead
        # of X(1)'s packets in the software DMA ring, shortening the prologue
        # critical chain.
        load_x(0)
        pools_of = {0: pools(0)}
        load_x(1)
        for g in range(NGRP):
            load_t2(g)
        t2_pools()
        for g in range(NGRP):
            if g + 1 < NGRP:
                pools_of[g + 1] = pools(g + 1)
            mx, bh = pools_of.pop(g)
            for pair in range(2):
                conv_sub(g, 0, pair, mx, bh)
            if g + 2 < NGRP:
                load_x(g + 2)
            for pair in range(2):
                conv_sub(g, 1, pair, mx, bh)
            flush_outputs()
            for pair in range(2):
                conv_c2(g, pair)
        flush_outputs(final=True)

    nc.compile()
    return nc


def kernel(input_tensor, conv_w, conv_b):
    input_tensor = np.ascontiguousarray(np.asarray(input_tensor, dtype=np.float32))
    conv_w = np.asarray(conv_w, dtype=np.float32)
    conv_b = np.asarray(conv_b, dtype=np.float32)

    if "nc" not in _cache:
        _cache["nc"] = _build_program()
    nc = _cache["nc"]

    stack = _build_stack(conv_w)
    bias_vec = np.full((128, 1), conv_b[0], dtype=np.float32)
    in_maps = [
        {"x": input_tensor[i], "cst": stack, "bias": bias_vec}
        for i in range(N_CORES)
    ]
    res = run_bass_kernel_spmd(nc, in_maps, list(range(N_CORES)))
    out = np.stack([res.results[i]["out"] for i in range(N_CORES)], axis=0)
    return out.astype(np.float32)


if __name__ == "__main__":
    rng = np.random.default_rng(0)
    x = rng.standard_normal((B, CTOT, H, W), dtype=np.float32)
    cw = rng.uniform(-0.1, 0.1, (1, 6, 3, 3)).astype(np.float32)
    cb = np.array([0.01], dtype=np.float32)
    o = kernel(x, cw, cb)
    print(o.shape, o.dtype)


# revision 35
# speedup vs baseline: 1.1854x; 1.1854x over previous
"""Trainium2 Bass kernel for nn_CBAMSpaceMask (CBAM spatial mask over T timestep blocks).

Math per timestep block t (3 channels):
  mx_c = maxpool3x3(x_c)          (stride 1, -inf pad == replicate pad)
  av_c = avgpool3x3(x_c)/9        (zero pad, count_include_pad)
  y_t  = sum_c wM_c * mx_c + wA_c * av_c + b   (3x3 conv, zero pad)
  out[3t+c] = sigmoid(leakyrelu(y_t))          (broadcast over c)

v2 design (per core = 1 batch element, pure data parallel over batch):
  - input loaded from HBM exactly once (f32->bf16 cast DMA, gpsimd SWDGE);
    row-shifted U/D tiles for the vertical max built by SBUF->SBUF
    partition-shifted DMA copies + clamp edges (edge pool rows have zero
    conv coefficients; clamps only keep values finite)
  - groups of 12 planes (= 4 timesteps); tiles [128, 12, 256] bf16
  - pools: vertical max (2 DVE ops), horizontal 3-tap max (2 DVE ops),
    horizontal box sum for the avg path (2 gpsimd ops); vertical box sum
    folded into the conv operator (op @ Bv)
  - conv: banded-Toeplitz matmuls on PE, timestep-PAIRED rhs (planes
    {c, c+3} via stride-3 free slice) -> N=2x254, psum [124, 2, 256]
  - last-8-rows chunk: rows 246..255 packed per-group at partition bases
    0/32/64/96; kw taps folded into K=30 stacked matmuls using L/R
    column-shifted copies of the pooled tiles
  - epilogue: ACT Identity(+bias) -> DVE leaky=max(v,.01v) -> ACT Sigmoid
    -> broadcast (0-stride) DMA writes each mask to its 3 channel planes
"""
import sys

sys.path.insert(0, "/opt/trn_rl_repo")

import numpy as np
import ml_dtypes
from contextlib import ExitStack

import concourse.bass as bass
import concourse.tile as tile
from concourse import bacc, mybir
from concourse.bass_utils import run_bass_kernel_spmd

F32 = mybir.dt.float32
BF16 = mybir.dt.bfloat16

B, CTOT, H, W = 8, 48, 256, 256
T = 16
N_CORES = 8
NGRP = 4            # groups of 12 planes = 4 timesteps
GP = 12             # planes per group
# main chunk geometry: y rows [m0,m1) from x rows [r0,r1)
SUBS = [(0, 124, 0, 128), (124, 248, 122, 250)]
C2 = (248, 256, 246, 256)   # last-8-rows chunk
NMAIN = 2 * 3 * 3 * 2       # path, c, kw, sub
NC2 = 2 * 3                 # path, c (kw folded into K=30)
NMAT = NMAIN + NC2

_cache = {}


def _build_stack(conv_w):
    """lhsT stack [128, NMAT, 128] bf16.

    mats 0..35: main-sub ops, idx = ((path*3 + c)*3 + kw)*2 + sub,
      lhsT = op[m0:m1, r0:r1].T  ([K=128, M=124])
    mats 36..41: chunk-2 stacked ops, idx = 36 + path*3 + c,
      [K=30, M=8]: K blocks of 10 rows for kw = 1 (center), 0, 2,
      replicated at partition bases 0/32/64/96.
    """
    w = conv_w[0].astype(np.float64)  # [6, 3, 3]
    Bv = np.zeros((H, H))
    for i in (-1, 0, 1):
        Bv += np.eye(H, k=i)
    stack = np.zeros((128, NMAT, 128), dtype=np.float64)

    def band_op(path, c, kw):
        op = np.zeros((H, H))
        k2d = w[2 * c] if path == 0 else w[2 * c + 1]
        for kh in range(3):
            op += k2d[kh, kw] * np.eye(H, k=kh - 1)
        if path == 1:
            op = (op @ Bv) / 9.0
        return op

    for path in range(2):
        for c in range(3):
            for kw in range(3):
                op = band_op(path, c, kw)
                for sub, (m0, m1, r0, r1) in enumerate(SUBS):
                    mat = ((path * 3 + c) * 3 + kw) * 2 + sub
                    lhsT = op[m0:m1, r0:r1].T  # [K, M]
                    K, M = lhsT.shape
                    stack[:K, mat, :M] = lhsT
            # chunk 2: kw-stacked [30, 8]
            mat = NMAIN + path * 3 + c
            m0, m1, r0, r1 = C2
            for kwi, kw in enumerate((1, 0, 2)):
                lhsT = band_op(path, c, kw)[m0:m1, r0:r1].T  # [10, 8]
                for base in (0, 32, 64, 96):
                    stack[base + 10 * kwi:base + 10 * kwi + 10, mat, :8] = lhsT
    return stack.astype(ml_dtypes.bfloat16)


def _mat_main(path, c, kw, sub):
    return ((path * 3 + c) * 3 + kw) * 2 + sub


def _mat_c2(path, c):
    return NMAIN + path * 3 + c


def _build_program():
    nc = bacc.Bacc("TRN2", target_bir_lowering=False, debug=False, enable_asserts=False)
    x_ap = nc.dram_tensor("x", [CTOT, H, W], F32, kind="ExternalInput").ap()
    cst_ap = nc.dram_tensor("cst", [128, NMAT, 128], BF16, kind="ExternalInput").ap()
    bias_ap = nc.dram_tensor("bias", [128, 1], F32, kind="ExternalInput").ap()
    # bf16 output: halves the output DMA volume through the software queue;
    # sigmoid outputs lie in (0,1) so bf16 quantization error (~0.4% rel) is
    # far inside the accuracy budget. Host upcasts to f32.
    out_ap = nc.dram_tensor("out", [CTOT, H, W], BF16, kind="ExternalOutput").ap()

    MAXOP = mybir.AluOpType.max
    ADDOP = mybir.AluOpType.add

    with tile.TileContext(nc) as tc, ExitStack() as ctx:
        const_pool = ctx.enter_context(tc.tile_pool(name="const", bufs=1))
        psum_pool = ctx.enter_context(tc.tile_pool(name="psum", bufs=6, space="PSUM"))
        epi_pool = ctx.enter_context(tc.tile_pool(name="epi", bufs=2))
        sg_pool = ctx.enter_context(tc.tile_pool(name="sg", bufs=10))
        t2_pool = ctx.enter_context(tc.tile_pool(name="t2", bufs=1))
        x_pool = ctx.enter_context(tc.tile_pool(name="xload", bufs=3))
        ud_pool = ctx.enter_context(tc.tile_pool(name="ud", bufs=2))
        mxbh_pool = ctx.enter_context(tc.tile_pool(name="mxbh", bufs=2))

        cst = const_pool.tile([128, NMAT, 128], BF16, tag="cst")
        nc.sync.dma_start(out=cst[:], in_=cst_ap)
        bias = const_pool.tile([128, 1], F32, tag="bias")
        nc.sync.dma_start(out=bias[:], in_=bias_ap)

        # ---- t2 tiles: rows 246..255 of group g at partitions 32g..32g+9,
        # planes 12g..12g+11 in the free dim. MX/BH are padded to 258 cols
        # (data at cols 1..256, zero pads) and also hold R/L column-shifted
        # copies at partition offsets +10 / +20 (kw-folded K=30).
        WP = W + 2
        T2X = t2_pool.tile([128, GP, W], BF16, tag="t2x")
        T2U = t2_pool.tile([128, GP, W], BF16, tag="t2u")
        T2D = t2_pool.tile([128, GP, W], BF16, tag="t2d")
        T2MX = t2_pool.tile([128, GP, WP], BF16, tag="t2mx")
        T2BH = t2_pool.tile([128, GP, WP], BF16, tag="t2bh")
        # zero-fill so gap partitions / pad columns stay finite zeros
        nc.vector.memzero(T2X[:])
        nc.vector.memzero(T2U[:])
        nc.vector.memzero(T2D[:])
        nc.vector.memzero(T2MX[:])
        nc.vector.memzero(T2BH[:])

        # Both row-subs share one tile: X[:, s] holds x rows r0s..r0s+127 of
        # the group's 12 planes (sub 0: rows 0..127, sub 1: rows 122..249).
        x_tiles = {}

        def load_x(g):
            """HBM->SBUF cast loads for group g (issue 2 groups ahead)."""
            src = x_ap[GP * g:GP * g + GP]
            X = x_pool.tile([128, 2, GP, W], BF16, tag="x")
            for sub, (m0, m1, r0, r1) in enumerate(SUBS):
                nc.gpsimd.dma_start(out=X[:, sub],
                                    in_=src[:, r0:r1, :].transpose([1, 0, 2]))
            x_tiles[g] = X

        def pools(g):
            """U/D partition-shifted copies + pools for both subs of g."""
            X = x_tiles.pop(g)
            U = ud_pool.tile([128, 2, GP, W], BF16, tag="u")
            D = ud_pool.tile([128, 2, GP, W], BF16, tag="d")
            # U[p] = X[p+1], D[p] = X[p-1]; clamps keep edge rows finite
            # (their conv coefficients are zero). gpsimd = software DGE:
            # packets spread across all 16 DMA engines (hardware DGE queues
            # pin SBUF->SBUF traffic to a single engine).
            nc.gpsimd.dma_start(out=U[0:127], in_=X[1:128])
            nc.gpsimd.dma_start(out=D[1:128], in_=X[0:127])
            nc.sync.dma_start(out=U[127:128], in_=X[127:128])
            nc.sync.dma_start(out=D[0:1], in_=X[0:1])
            # mx/bh padded: data at cols 1..256, cols 0/257 stay zero
            mx = mxbh_pool.tile([128, 2, GP, WP], BF16, tag="mx")
            bh = mxbh_pool.tile([128, 2, GP, WP], BF16, tag="bh")
            nc.vector.memset(mx[:, :, :, 0:1], 0)
            nc.vector.memset(mx[:, :, :, 257:258], 0)
            nc.vector.memset(bh[:, :, :, 0:1], 0)
            nc.vector.memset(bh[:, :, :, 257:258], 0)
            # bh first: it only needs X, so DVE proceeds while the U/D
            # shift DMAs are still in flight
            nc.vector.tensor_tensor(out=bh[:, :, :, 1:256], in0=X[:, :, :, 0:255],
                                    in1=X[:, :, :, 1:256], op=ADDOP)
            nc.vector.tensor_copy(bh[:, :, :, 256:257], X[:, :, :, 255:256])
            nc.vector.tensor_tensor(out=bh[:, :, :, 2:257], in0=bh[:, :, :, 2:257],
                                    in1=X[:, :, :, 0:255], op=ADDOP)
            # vertical 3-row max (DVE), in place into U
            vx = U
            nc.vector.tensor_tensor(out=vx[:], in0=U[:], in1=D[:], op=MAXOP)
            nc.vector.tensor_tensor(out=vx[:], in0=vx[:], in1=X[:], op=MAXOP)
            # horizontal 3-tap max (DVE) into padded mx
            nc.vector.tensor_tensor(out=mx[:, :, :, 1:256], in0=vx[:, :, :, 0:255],
                                    in1=vx[:, :, :, 1:256], op=MAXOP)
            nc.vector.tensor_copy(mx[:, :, :, 256:257], vx[:, :, :, 255:256])
            nc.vector.tensor_tensor(out=mx[:, :, :, 2:257], in0=mx[:, :, :, 2:257],
                                    in1=vx[:, :, :, 0:255], op=MAXOP)
            return mx, bh

        def load_t2(g):
            b = 32 * g
            src = x_ap[GP * g:GP * g + GP]
            m0, m1, r0, r1 = C2
            nc.gpsimd.dma_start(out=T2X[b:b + 10],
                                in_=src[:, r0:r1, :].transpose([1, 0, 2]))
            nc.sync.dma_start(out=T2U[b:b + 9], in_=T2X[b + 1:b + 10])
            nc.sync.dma_start(out=T2U[b + 9:b + 10], in_=T2X[b + 9:b + 10])
            nc.sync.dma_start(out=T2D[b + 1:b + 10], in_=T2X[b:b + 9])
            nc.sync.dma_start(out=T2D[b:b + 1], in_=T2X[b:b + 1])

        def t2_pools():
            """Pools over the whole packed tile (all 4 groups at once)."""
            vx = T2U
            nc.vector.tensor_tensor(out=vx[:], in0=T2U[:], in1=T2D[:], op=MAXOP)
            nc.vector.tensor_tensor(out=vx[:], in0=vx[:], in1=T2X[:], op=MAXOP)
            nc.vector.tensor_tensor(out=T2MX[:, :, 1:256], in0=vx[:, :, 0:255],
                                    in1=vx[:, :, 1:256], op=MAXOP)
            nc.vector.tensor_copy(T2MX[:, :, 256:257], vx[:, :, 255:256])
            nc.vector.tensor_tensor(out=T2MX[:, :, 2:257], in0=T2MX[:, :, 2:257],
                                    in1=vx[:, :, 0:255], op=MAXOP)
            nc.vector.tensor_tensor(out=T2BH[:, :, 1:256], in0=T2X[:, :, 0:255],
                                    in1=T2X[:, :, 1:256], op=ADDOP)
            nc.vector.tensor_copy(T2BH[:, :, 256:257], T2X[:, :, 255:256])
            nc.vector.tensor_tensor(out=T2BH[:, :, 2:257], in0=T2BH[:, :, 2:257],
                                    in1=T2X[:, :, 0:255], op=ADDOP)
            # R/L column-shifted copies into partition blocks +10 / +20.
            # All matmul rhs windows read cols 1..256 of their block:
            #   block +10 pairs kw=0 (needs P[w-1]): dest col j <- data col j-1
            #   block +20 pairs kw=2 (needs P[w+1]): dest col j <- data col j+1
            # Pad columns supply the conv zero padding.
            for g in range(NGRP):
                b = 32 * g
                for tl in (T2MX, T2BH):
                    nc.sync.dma_start(out=tl[b + 10:b + 20, :, 2:257],
                                      in_=tl[b:b + 10, :, 1:256])
                    nc.sync.dma_start(out=tl[b + 20:b + 30, :, 1:256],
                                      in_=tl[b:b + 10, :, 2:257])

        KW_ORDER = [(path, c, kw)
                    for c in range(3) for path in range(2) for kw in (1, 0, 2)]
        KW_ORDER.sort(key=lambda pck: 0 if pck[2] == 1 else 1)

        out_ready = []   # sigmoids surely complete: safe to issue triggers
        out_recent = []  # freshly issued sigmoids: age one checkpoint first

        def epilogue(ps, M, p0, m0, m1):
            """psum [M, 2, W] planar -> sigmoid(leaky(v + bias)); output DMA
            issue is DEFERRED (gpsimd queue is in-order: an output trigger
            waiting on its sigmoid would stall later load triggers)."""
            v = epi_pool.tile([128, 2, W], F32, tag="epiv")
            nc.scalar.activation(v[0:M], ps[0:M],
                                 mybir.ActivationFunctionType.Identity,
                                 bias=bias[0:M], scale=1.0)
            lk = epi_pool.tile([128, 2, W], F32, tag="epil")
            nc.vector.scalar_tensor_tensor(out=lk[0:M], in0=v[0:M], scalar=0.01,
                                           in1=v[0:M], op0=mybir.AluOpType.mult,
                                           op1=MAXOP)
            sg = sg_pool.tile([128, 2, W], BF16, tag="epis")
            nc.scalar.activation(sg[0:M], lk[0:M],
                                 mybir.ActivationFunctionType.Sigmoid)
            out_recent.append((sg, M, p0, m0, m1))

        def flush_outputs(final=False):
            for sg, M, p0, m0, m1 in out_ready:
                for j in range(2):
                    dst = out_ap[p0 + 3 * j:p0 + 3 * j + 3, m0:m1, :] \
                        .transpose([1, 0, 2])
                    src = sg[0:M, j:j + 1, :].to_broadcast([M, 3, W])
                    nc.gpsimd.dma_start(out=dst, in_=src)
            out_ready.clear()
            out_ready.extend(out_recent)
            out_recent.clear()
            if final and out_ready:
                flush_outputs()

        def conv_sub(g, sub, pair, mx, bh):
            """Main-chunk accumulation for timestep pair `pair` of group g.

            All 18 matmuls are full-width N=512: the kw tap s reads the
            padded rhs window cols (1+s)..(256+s), whose zero pads supply
            the conv zero padding.
            """
            m0, m1, r0, r1 = SUBS[sub]
            M, K = m1 - m0, r1 - r0
            ps = psum_pool.tile([128, 2, W], F32, tag="ps")
            n = len(KW_ORDER)
            for i, (path, c, kw) in enumerate(KW_ORDER):
                s = kw - 1
                mat = _mat_main(path, c, kw, sub)
                srcs = mx if path == 0 else bh
                pl = 6 * pair + c
                rhs = srcs[0:K, sub, pl:pl + 4:3, 1 + s:257 + s]
                nc.tensor.matmul(ps[0:M], cst[0:K, mat, 0:M], rhs,
                                 start=(i == 0), stop=(i == n - 1))
            p0 = GP * g + 6 * pair
            epilogue(ps, M, p0, m0, m1)

        def conv_c2(g, pair):
            """Last-8-rows accumulation (kw-folded, K=30) for a pair."""
            m0, m1, r0, r1 = C2
            M = m1 - m0
            b = 32 * g
            ps = psum_pool.tile([128, 2, W], F32, tag="ps")
            idx = 0
            for path in range(2):
                for c in range(3):
                    mat = _mat_c2(path, c)
                    src = T2MX if path == 0 else T2BH
                    pl = 6 * pair + c
                    rhs = src[b:b + 30, pl:pl + 4:3, 1:257]
                    nc.tensor.matmul(ps[0:M], cst[b:b + 30, mat, 0:M], rhs,
                                     start=(idx == 0), stop=(idx == NC2 - 1),
                                     tile_position=(b, 0))
                    idx += 1
            p0 = GP * g + 6 * pair
            epilogue(ps, M, p0, m0, m1)

        # ---- schedule: X loads run 2 groups ahead; pools for g+1 are issued
        # at the TOP of iteration g, ahead of the epilogue STTs in the DVE
        # queue, so they execute while PE runs group g's convs. Output
        # triggers age through two checkpoints before issue so no gpsimd
        # trigger ever waits on an incomplete producer.
        load_x(0)
        load_x(1)
        pools_of = {0: pools(0)}
        for g in range(NGRP):
            load_t2(g)
        t2_pools()
        for g in range(NGRP):
            if g + 1 < NGRP:
                pools_of[g + 1] = pools(g + 1)
            mx, bh = pools_of.pop(g)
            for pair in range(2):
                conv_sub(g, 0, pair, mx, bh)
            if g + 2 < NGRP:
                load_x(g + 2)
            for pair in range(2):
                conv_sub(g, 1, pair, mx, bh)
            flush_outputs()
            for pair in range(2):
                conv_c2(g, pair)
        flush_outputs(final=True)

    nc.compile()
    return nc


def kernel(input_tensor, conv_w, conv_b):
    input_tensor = np.ascontiguousarray(np.asarray(input_tensor, dtype=np.float32))
    conv_w = np.asarray(conv_w, dtype=np.float32)
    conv_b = np.asarray(conv_b, dtype=np.float32)

    if "nc" not in _cache:
        _cache["nc"] = _build_program()
    nc = _cache["nc"]

    stack = _build_stack(conv_w)
    bias_vec = np.full((128, 1), conv_b[0], dtype=np.float32)
    in_maps = [
        {"x": input_tensor[i], "cst": stack, "bias": bias_vec}
        for i in range(N_CORES)
    ]
    res = run_bass_kernel_spmd(nc, in_maps, list(range(N_CORES)))
    out = np.stack([res.results[i]["out"] for i in range(N_CORES)], axis=0)
    return out.astype(np.float32)


if __name__ == "__main__":
    rng = np.random.default_rng(0)
    x = rng.standard_normal((B, CTOT, H, W), dtype=np.float32)
    cw = rng.uniform(-0.1, 0.1, (1, 6, 3, 3)).astype(np.float32)
    cb = np.array([0.01], dtype=np.float32)
    o = kernel(x, cw, cb)
    print(o.shape, o.dtype)
